# revision 9
# baseline (speedup 1.0000x reference)
"""Trainium2 Bass kernel for the pairwise-distance masked log-sum loss.

Reference math (N=8192 points, E=49152 edges):
    dist[i,j] = |p_i - p_j|^2 + 1e-8
    mask      = (dist <= 0.25), edges (both directions) and diagonal zeroed
    loss      = sum(-log(dist) * mask)

Device strategy (8 NeuronCores, SPMD, row-sharded):
    dist[i,j] = w_i . u_j with K=5:
        w_i = (-2x_i, -2y_i, -2z_i, 1, |p_i|^2)      (matmul lhsT)
        u_j = (x_j,  y_j,  z_j,  |p_j|^2 + 1e-8, 1)  (matmul rhs)
    TensorE computes dist into PSUM, ScalarE computes y = ln(dist) (bf16),
    VectorE computes sum(y * (y <= ln .25)) per partition via the fused
    scalar_tensor_tensor accumulate.  The diagonal is pushed out of the
    mask on-device by accumulating +10*I onto the diagonal 128x128 block
    (second matmul with an identity lhsT), so ln never sees dist ~ 1e-8
    garbage.  Each core gets its column array rolled by its row offset so
    the diagonal block lands at the same column index on every core
    (identical SPMD program).

    Host side: sum the per-core/per-partition accumulators, then add back
    the (tiny, ~100k) edge-pair contributions the reference masks out.
"""

import os

import numpy as np

N = 8192
NCORES = 8
ROWS_PER_CORE = N // NCORES  # 1024
ROW_TILE = 128
ROW_TILES = ROWS_PER_CORE // ROW_TILE  # 8
COL_CHUNK = 512  # one PSUM bank
GROUP_COLS = 2048  # 4 banks per ACT/DVE group
GROUPS = N // GROUP_COLS  # 4
EPS = 1e-8
THR2 = 0.25
LN_THR = float(np.log(0.25))  # -1.3862943611198906

# Set USE_FP32R=1 to run the distance matmuls in float32r (full PE rate,
# reduced internal precision) instead of float32 (4x slower, exact).
USE_FP32R = os.environ.get("KERNEL_FP32R", "1") == "1"

LAST_RESULT = {}


def _build_vectors(pred_pos: np.ndarray):
    """Host-side prep of the K=5 w/u vectors, float32."""
    p = np.asarray(pred_pos, dtype=np.float32)
    x, y, z = p[:, 0], p[:, 1], p[:, 2]
    sq = x * x + y * y + z * z  # float32
    wt = np.stack([-2.0 * x, -2.0 * y, -2.0 * z, np.ones_like(x), sq]).astype(
        np.float32
    )  # [5, N]
    u = np.stack([x, y, z, sq + np.float32(EPS), np.ones_like(x)]).astype(
        np.float32
    )  # [5, N]
    return wt, u


def _edge_correction(pred_pos: np.ndarray, edges: np.ndarray) -> float:
    """sum of ln(dist) over the unique directed edge pairs (a != b) that are
    inside the threshold -- the pairs the reference zeroes but the device
    sum includes."""
    p = np.asarray(pred_pos, dtype=np.float32)
    e = np.asarray(edges, dtype=np.int64)
    e = e[e[:, 0] != e[:, 1]]
    z = np.concatenate([e, e[:, ::-1]], axis=0)
    z = np.unique(z, axis=0)
    d = p[z[:, 0]] - p[z[:, 1]]
    dist = (d * d).sum(axis=1, dtype=np.float32) + np.float32(EPS)
    m = dist <= np.float32(THR2)
    return float(np.log(dist[m].astype(np.float64)).sum())


def _build_program():
    import concourse.bass as bass
    import concourse.tile as tile
    from concourse import mybir

    f32 = mybir.dt.float32
    mm_dt = mybir.dt.float32r if USE_FP32R else mybir.dt.float32
    bf16 = mybir.dt.bfloat16

    nc = bass.Bass("TRN2", target_bir_lowering=False, debug=False, num_devices=NCORES)

    # pack (wt | u) and (id1 | id10) so each needs only one DMA -> the
    # consuming matmuls carry few distinct semaphore waits (walrus caps the
    # sync-wait commands per PE instruction).
    wtu_d = nc.dram_tensor(
        "wtu", [5, ROWS_PER_CORE + N], f32, kind="ExternalInput"
    ).ap()
    idid_d = nc.dram_tensor("idid", [128, 256], f32, kind="ExternalInput").ap()
    acc_d = nc.dram_tensor(
        "acc", [128, ROW_TILES * GROUPS], f32, kind="ExternalOutput"
    ).ap()

    from contextlib import ExitStack

    with tile.TileContext(nc) as tc, ExitStack() as ctx:
        singles = ctx.enter_context(tc.tile_pool(name="singles", bufs=1))
        psums = ctx.enter_context(tc.tile_pool(name="psums", bufs=2, space="PSUM"))
        # one y buffer per group: no slot reuse, so the ACT never needs a
        # second (DVE slot-release) wait -- walrus allows only one sync wait
        # per compute instruction.
        ys = ctx.enter_context(tc.tile_pool(name="ys", bufs=ROW_TILES * GROUPS))
        scraps = ctx.enter_context(tc.tile_pool(name="scraps", bufs=2))

        wtu_s = singles.tile([5, ROWS_PER_CORE + N], f32)
        nc.sync.dma_start(out=wtu_s, in_=wtu_d)
        idid_s = singles.tile([128, 256], f32)
        nc.sync.dma_start(out=idid_s, in_=idid_d)
        acc_s = singles.tile([128, ROW_TILES * GROUPS], f32)

        wtu_mm = wtu_s.bitcast(mm_dt) if USE_FP32R else wtu_s
        idid_mm = idid_s.bitcast(mm_dt) if USE_FP32R else idid_s
        wt_mm = wtu_mm[:, :ROWS_PER_CORE]
        u_mm = wtu_mm[:, ROWS_PER_CORE:]
        id1_mm = idid_mm[:, :128]
        id10_mm = idid_mm[:, 128:]

        for r in range(ROW_TILES):
            lhsT = wt_mm[:, r * ROW_TILE : (r + 1) * ROW_TILE]  # [5, 128]
            for g in range(GROUPS):
                psum_t = psums.tile([128, GROUP_COLS], f32)
                for k in range(GROUP_COLS // COL_CHUNK):
                    col0 = g * GROUP_COLS + k * COL_CHUNK
                    # local diagonal block for row-tile r sits at columns
                    # [r*128, r*128+128) thanks to the per-core column roll
                    diag0 = r * ROW_TILE
                    has_diag = col0 <= diag0 < col0 + COL_CHUNK
                    nc.tensor.matmul(
                        out=psum_t[:, k * COL_CHUNK : (k + 1) * COL_CHUNK],
                        lhsT=lhsT,
                        rhs=u_mm[:, col0 : col0 + COL_CHUNK],
                        start=True,
                        stop=not has_diag,
                    )
                    if has_diag:
                        off = k * COL_CHUNK + (diag0 - col0)
                        nc.tensor.matmul(
                            out=psum_t[:, off : off + 128],
                            lhsT=id1_mm,
                            rhs=id10_mm,
                            start=False,
                            stop=True,
                        )
                y_t = ys.tile([128, GROUP_COLS], bf16)
                nc.scalar.activation(
                    out=y_t,
                    in_=psum_t,
                    func=mybir.ActivationFunctionType.Ln,
                )
                scrap_t = scraps.tile([128, GROUP_COLS], bf16)
                idx = r * GROUPS + g
                nc.vector.scalar_tensor_tensor(
                    out=scrap_t,
                    in0=y_t,
                    scalar=LN_THR,
                    in1=y_t,
                    op0=mybir.AluOpType.is_le,
                    op1=mybir.AluOpType.mult,
                    accum_out=acc_s[:, idx : idx + 1],
                )

        nc.sync.dma_start(out=acc_d, in_=acc_s)

    _strip_self_waits(nc, mybir)
    return nc


# Walrus caps the sync-wait commands per instruction (1 for PE LW_STRUCT /
# DVE STT_STRUCT).  Tile conservatively emits same-engine self-waits (e.g.
# a matmul waiting on the PE semaphore) alongside the real cross-engine
# waits.  The compute engines are in-order -- an instruction can never run
# before its same-engine predecessors complete (DVE additionally drains its
# pipe between ops) -- so self-waits carry no information; drop them.
_SELF_WAIT_OPCODES = {
    "InstMatmult",
    "InstTensorScalarPtr",
    "InstActivation",
    "InstTensorTensor",
    "InstTensorReduce",
    "InstTensorCopy",
    "InstMemset",
}
_ENGINE_SEM_PREFIX = {
    "PE": "PE_",
    "ACT": "Activation_",
    "DVE": "DVE_",
    "POOL": "Pool_",
    "SP": "SP_",
}


def _strip_self_waits(nc, mybir):
    # Split multi-wait Drains (e.g. the kernel-tail SP drain waiting on all
    # procs) into a chain of single-wait drains: sequential drains each
    # waiting one semaphore are equivalent to one drain waiting on all.
    split_id = 0
    for fn in nc.m.functions:
        for bb in fn.blocks:
            idx = 0
            insts = bb.instructions
            while idx < len(insts):
                inst = insts[idx]
                si = inst.sync_info
                if (
                    type(inst).__name__ == "InstDrain"
                    and si is not None
                    and si.on_wait
                    and len(si.on_wait) > 1
                ):
                    waits = list(si.on_wait)
                    inst.sync_info = mybir.SyncInfo(
                        on_wait=[waits[-1]], on_update=si.on_update
                    )
                    for w in waits[:-1]:
                        nd = mybir.InstDrain(
                            name=f"I-drainsplit-{split_id}",
                            ins=[],
                            outs=[],
                            bass_is_fusable=False,
                        )
                        split_id += 1
                        nd.engine = inst.engine
                        nd.sync_info = mybir.SyncInfo(on_wait=[w], on_update=[])
                        insts.insert(idx, nd)
                        idx += 1
                idx += 1

    for fn in nc.m.functions:
        for bb in fn.blocks:
            for inst in bb.instructions:
                si = inst.sync_info
                if si is None or not si.on_wait or len(si.on_wait) < 2:
                    continue
                if type(inst).__name__ not in _SELF_WAIT_OPCODES:
                    continue
                eng = getattr(inst.engine, "name", str(inst.engine))
                prefix = None
                for k, v in _ENGINE_SEM_PREFIX.items():
                    if k in str(eng).upper():
                        prefix = v
                        break
                if prefix is None:
                    continue
                waits = list(si.on_wait)
                keep = [w for w in waits if not w.ant_name.startswith(prefix)]
                if len(keep) != len(waits) and keep:
                    inst.sync_info = mybir.SyncInfo(
                        on_wait=keep, on_update=si.on_update
                    )


def kernel(pred_pos: np.ndarray, edges: np.ndarray) -> np.ndarray:
    from concourse.bass_utils import run_bass_kernel_spmd

    wt, u = _build_vectors(pred_pos)

    idid = np.concatenate(
        [np.eye(128, dtype=np.float32), np.eye(128, dtype=np.float32) * 10.0], axis=1
    ).astype(np.float32)

    in_maps = []
    for c in range(NCORES):
        r0 = c * ROWS_PER_CORE
        wtu = np.concatenate(
            [wt[:, r0 : r0 + ROWS_PER_CORE], np.roll(u, -r0, axis=1)], axis=1
        ).astype(np.float32)
        in_maps.append({"wtu": np.ascontiguousarray(wtu), "idid": idid})

    nc = _build_program()
    trace = os.environ.get("KERNEL_TRACE", "0") == "1"
    res = run_bass_kernel_spmd(
        nc,
        in_maps,
        core_ids=list(range(NCORES)),
        trace=trace,
    )
    LAST_RESULT["exec_time_ns"] = res.exec_time_ns
    LAST_RESULT["trace"] = res.instructions_and_trace

    s = 0.0
    for r in res.results:
        s += r["acc"].astype(np.float64).sum()

    corr = _edge_correction(pred_pos, edges)
    loss = -s + corr
    return np.float32(loss)


# revision 13
# speedup vs baseline: 5.4346x; 5.4346x over previous
"""Trainium2 Bass kernel for the pairwise-distance masked log-sum loss.

Reference math (N=8192 points, E=49152 edges):
    dist[i,j] = |p_i - p_j|^2 + 1e-8
    mask      = (dist <= 0.25), edges (both directions) and diagonal zeroed
    loss      = sum(-log(dist) * mask)

Device strategy (8 NeuronCores, SPMD):
  * Points are sorted by x on the host.  Only pairs with |dx| < 0.5 can be
    inside the threshold, so each 128-row tile only needs its own blocks
    (intra-tile pairs) plus a forward window of sorted columns reaching
    x_max(tile) + 0.5.  Pairs outside the window contribute exactly 0.
    Forward windows count each inter-tile pair once; the host doubles that
    partial sum (dist is symmetric).  The 64 row tiles are dealt to the 8
    cores by snake order of window size, so every core runs an identical
    program on identically-shaped inputs (windows padded with far-away
    dummy points, which fall outside the threshold and contribute 0).
  * dist[i,j] = w_i . u_j with K=16 split-precision channels evaluated on
    the TensorE in float32r.  f32r is fp32 rounded to 11 explicit mantissa
    bits (measured round-to-nearest on hardware) at full bf16 PE rate; the
    host splits each coordinate c = ch + cl and |p|^2 = sqh + sql with
    rn11 so every channel is exactly representable and the product sum
    reconstructs |p_i - p_j|^2 + |p|^2-rounding (~1e-6) -- fp32-grade.
  * The diagonal 128x128 block has +10*I accumulated onto it by a second
    matmul (identity lhsT) so ln never sees the dist ~ 1e-8 diagonal.
  * ScalarE computes y = ln(dist) PSUM->SBUF (bf16), VectorE computes
    sum(y * (y <= ln .25)) per partition with the fused
    scalar_tensor_tensor accumulate.
  * Host: loss = -(S_diag + 2*S_windows) + 2*sum(ln dist) over the unique
    non-self edge pairs inside the threshold (the reference masks those
    out, the device sum includes them).
"""

import os

import numpy as np

N = 8192
NCORES = 8
ROW_TILE = 128
TILES = N // ROW_TILE  # 64
SLOTS = TILES // NCORES  # 8 row-tiles per core
KCH = 16  # split-precision channels
COL_CHUNK = 512  # one PSUM bank per matmul
GROUP_COLS = 2048  # ACT/DVE group = 4 PSUM banks
EPS = 1e-8
THR2 = 0.25
XWIN = 0.5
LN_THR = float(np.log(0.25))

USE_FP32R = os.environ.get("KERNEL_FP32R", "1") == "1"
ACC_SLOTS = 64

LAST_RESULT = {}


def _rn11(v: np.ndarray) -> np.ndarray:
    """Round f32/f64 values to 11 explicit mantissa bits (round-to-nearest)
    -- the measured float32r grid."""
    v64 = np.asarray(v, dtype=np.float64)
    m, e = np.frexp(v64)
    q = np.ldexp(np.round(np.ldexp(m, 12)) / (1 << 12), e)
    return q.astype(np.float32)


def _build_channels(pts: np.ndarray):
    """w [16, n] and u [16, n] channel vectors, f32r-grid values, such that
    sum_k w[k,i]*u[k,j] ~= |p_i - p_j|^2 (+eps folded into sql)."""
    c = np.asarray(pts, dtype=np.float32)
    ch = _rn11(c)
    cl = _rn11(c.astype(np.float64) - ch)
    rep = ch.astype(np.float64) + cl  # represented points
    sq = (rep * rep).sum(axis=1)  # f64, exact-ish
    sqh = _rn11(sq)
    sql = _rn11(sq - sqh)

    n = c.shape[0]
    w = np.empty((KCH, n), np.float32)
    u = np.empty((KCH, n), np.float32)
    for a in range(3):
        w[4 * a + 0] = -2.0 * ch[:, a]
        u[4 * a + 0] = ch[:, a]
        w[4 * a + 1] = -2.0 * ch[:, a]
        u[4 * a + 1] = cl[:, a]
        w[4 * a + 2] = -2.0 * cl[:, a]
        u[4 * a + 2] = ch[:, a]
        w[4 * a + 3] = -2.0 * cl[:, a]
        u[4 * a + 3] = cl[:, a]
    w[12] = sqh
    u[12] = 1.0
    w[13] = sql
    u[13] = 1.0
    w[14] = 1.0
    u[14] = sqh
    w[15] = 1.0
    u[15] = sql
    return w, u


def _host_prep(pred_pos: np.ndarray):
    """Sort, window, balance; build per-core in_maps and program meta."""
    p = np.asarray(pred_pos, dtype=np.float32)
    order = np.argsort(p[:, 0], kind="stable")
    ps = p[order]
    xs = ps[:, 0].astype(np.float64)

    w, u = _build_channels(ps)

    # forward window of each row tile
    wins = []
    for t in range(TILES):
        end = (t + 1) * ROW_TILE
        xmax = xs[end - 1]
        hi = int(np.searchsorted(xs, np.nextafter(xmax + XWIN, np.inf), side="right"))
        wins.append((end, max(hi - end, 0)))

    # snake-deal tiles (desc window size) to cores; slot s width = max in band
    rank = sorted(range(TILES), key=lambda t: -wins[t][1])
    assign = [[None] * SLOTS for _ in range(NCORES)]
    for s in range(SLOTS):
        band = rank[s * NCORES : (s + 1) * NCORES]
        cores = range(NCORES) if s % 2 == 0 else range(NCORES - 1, -1, -1)
        for t, c in zip(band, cores):
            assign[c][s] = t
    slot_w = []
    for s in range(SLOTS):
        wmax = max(wins[assign[c][s]][1] for c in range(NCORES))
        slot_w.append(int(np.ceil(wmax / ROW_TILE)) * ROW_TILE if wmax else 0)

    # dummy far-away point channels (outside any threshold window)
    wd, ud = _build_channels(np.array([[1000.0, 0.0, 0.0]], np.float32))

    # per-core packed input [KCH, SLOTS*128 (rowsW) | SLOTS*128 (rowsU) | windows]
    tot_w = sum(slot_w)
    width = SLOTS * ROW_TILE * 2 + tot_w
    in_maps = []
    idid = np.concatenate(
        [np.eye(128, dtype=np.float32), np.eye(128, dtype=np.float32) * 10.0], axis=1
    ).astype(np.float32)
    for c in range(NCORES):
        inp = np.empty((KCH, width), np.float32)
        for s in range(SLOTS):
            t = assign[c][s]
            r0 = t * ROW_TILE
            inp[:, s * ROW_TILE : (s + 1) * ROW_TILE] = w[:, r0 : r0 + ROW_TILE]
            inp[:, (SLOTS + s) * ROW_TILE : (SLOTS + s + 1) * ROW_TILE] = u[
                :, r0 : r0 + ROW_TILE
            ]
        off = SLOTS * ROW_TILE * 2
        for s in range(SLOTS):
            t = assign[c][s]
            end, wlen = wins[t]
            take = min(wlen, slot_w[s])
            inp[:, off : off + take] = u[:, end : end + take]
            if take < slot_w[s]:
                inp[:, off + take : off + slot_w[s]] = ud
            off += slot_w[s]
        in_maps.append({"inp": np.ascontiguousarray(inp), "idid": idid})

    meta = {"slot_w": slot_w, "width": width}
    return in_maps, meta


def _edge_correction(pred_pos: np.ndarray, edges: np.ndarray) -> float:
    """sum of ln(dist) over unique unordered non-self edge pairs inside the
    threshold (each such pair appears exactly twice in the device sum)."""
    p = np.asarray(pred_pos, dtype=np.float32)
    e = np.asarray(edges, dtype=np.int64)
    e = e[e[:, 0] != e[:, 1]]
    e = np.sort(e, axis=1)
    e = np.unique(e, axis=0)
    d = p[e[:, 0]] - p[e[:, 1]]
    dist = (d * d).sum(axis=1, dtype=np.float32) + np.float32(EPS)
    m = dist <= np.float32(THR2)
    return float(np.log(dist[m].astype(np.float64)).sum())


def _build_program(meta):
    import concourse.bass as bass
    import concourse.tile as tile
    from concourse import mybir
    from contextlib import ExitStack

    f32 = mybir.dt.float32
    bf16 = mybir.dt.bfloat16
    in_dt = mybir.dt.float32r if USE_FP32R else f32

    slot_w = meta["slot_w"]
    width = meta["width"]

    nc = bass.Bass("TRN2", target_bir_lowering=False, debug=False, num_devices=NCORES)
    inp_d = nc.dram_tensor("inp", [KCH, width], in_dt, kind="ExternalInput").ap()
    idid_d = nc.dram_tensor("idid", [128, 256], in_dt, kind="ExternalInput").ap()
    acc_d = nc.dram_tensor("acc", [128, ACC_SLOTS], f32, kind="ExternalOutput").ap()

    # how many ACT/DVE groups in total (for y-buffer count: no slot reuse)
    n_groups = SLOTS + sum(
        (wl + GROUP_COLS - 1) // GROUP_COLS for wl in slot_w if wl
    )
    assert n_groups <= ACC_SLOTS

    with tile.TileContext(nc) as tc, ExitStack() as ctx:
        singles = ctx.enter_context(tc.tile_pool(name="singles", bufs=1))
        psums = ctx.enter_context(tc.tile_pool(name="psums", bufs=2, space="PSUM"))
        ys = ctx.enter_context(tc.tile_pool(name="ys", bufs=n_groups))
        scraps = ctx.enter_context(tc.tile_pool(name="scraps", bufs=2))

        inp_s = singles.tile([KCH, width], in_dt)
        nc.sync.dma_start(out=inp_s, in_=inp_d)
        idid_s = singles.tile([128, 256], in_dt)
        nc.sync.dma_start(out=idid_s, in_=idid_d)
        acc_s = singles.tile([128, ACC_SLOTS], f32)

        id1 = idid_s[:, :128]
        id10 = idid_s[:, 128:]

        # acc layout: columns [0, SLOTS) = diagonal blocks (each ordered
        # intra pair once), columns [SLOTS, ...) = window groups (each
        # unordered inter pair once; host doubles them).
        win_acc = [SLOTS]

        def reduce_group(psum_t, cols, acc_idx):
            y_t = ys.tile([128, GROUP_COLS], bf16, tag="y")
            nc.scalar.activation(
                out=y_t[:, :cols],
                in_=psum_t[:, :cols],
                func=mybir.ActivationFunctionType.Ln,
            )
            scrap_t = scraps.tile([128, GROUP_COLS], bf16, tag="scrap")
            nc.vector.scalar_tensor_tensor(
                out=scrap_t[:, :cols],
                in0=y_t[:, :cols],
                scalar=LN_THR,
                in1=y_t[:, :cols],
                op0=mybir.AluOpType.is_le,
                op1=mybir.AluOpType.mult,
                accum_out=acc_s[:, acc_idx : acc_idx + 1],
            )

        win_off = SLOTS * ROW_TILE * 2
        for s in range(SLOTS):
            lhsT = inp_s[:, s * ROW_TILE : (s + 1) * ROW_TILE]
            # diagonal block + 10*I
            psum_t = psums.tile([128, GROUP_COLS], f32, tag="ps")
            nc.tensor.matmul(
                out=psum_t[:, :ROW_TILE],
                lhsT=lhsT,
                rhs=inp_s[:, (SLOTS + s) * ROW_TILE : (SLOTS + s + 1) * ROW_TILE],
                start=True,
                stop=False,
            )
            nc.tensor.matmul(
                out=psum_t[:, :ROW_TILE],
                lhsT=id1,
                rhs=id10,
                start=False,
                stop=True,
            )
            reduce_group(psum_t, ROW_TILE, s)

            # forward window
            wl = slot_w[s]
            done = 0
            while done < wl:
                cols = min(GROUP_COLS, wl - done)
                psum_t = psums.tile([128, GROUP_COLS], f32, tag="ps")
                for k0 in range(0, cols, COL_CHUNK):
                    kw = min(COL_CHUNK, cols - k0)
                    c0 = win_off + done + k0
                    nc.tensor.matmul(
                        out=psum_t[:, k0 : k0 + kw],
                        lhsT=lhsT,
                        rhs=inp_s[:, c0 : c0 + kw],
                        start=True,
                        stop=True,
                    )
                reduce_group(psum_t, cols, win_acc[0])
                win_acc[0] += 1
                done += cols
            win_off += wl
        assert win_acc[0] <= ACC_SLOTS

        nc.sync.dma_start(out=acc_d, in_=acc_s)

    _strip_self_waits(nc, mybir)
    return nc


_SELF_WAIT_OPCODES = {
    "InstMatmult",
    "InstTensorScalarPtr",
    "InstActivation",
    "InstTensorTensor",
    "InstTensorReduce",
    "InstTensorCopy",
    "InstMemset",
}
_ENGINE_SEM_PREFIX = {
    "PE": "PE_",
    "ACT": "Activation_",
    "DVE": "DVE_",
    "POOL": "Pool_",
    "SP": "SP_",
}


def _strip_self_waits(nc, mybir):
    """Walrus caps sync-wait commands per instruction (1 for PE/DVE compute
    structs).  Tile conservatively emits same-engine self-waits alongside
    the real cross-engine waits; the engines are in-order so self-waits are
    vacuous -- drop them.  Multi-wait Drains (kernel tail) are split into
    chains of single-wait drains (sequentially equivalent)."""
    split_id = 0
    for fn in nc.m.functions:
        for bb in fn.blocks:
            idx = 0
            insts = bb.instructions
            while idx < len(insts):
                inst = insts[idx]
                si = inst.sync_info
                if (
                    type(inst).__name__ == "InstDrain"
                    and si is not None
                    and si.on_wait
                    and len(si.on_wait) > 1
                ):
                    waits = list(si.on_wait)
                    inst.sync_info = mybir.SyncInfo(
                        on_wait=[waits[-1]], on_update=si.on_update
                    )
                    for w in waits[:-1]:
                        nd = mybir.InstDrain(
                            name=f"I-drainsplit-{split_id}",
                            ins=[],
                            outs=[],
                            bass_is_fusable=False,
                        )
                        split_id += 1
                        nd.engine = inst.engine
                        nd.sync_info = mybir.SyncInfo(on_wait=[w], on_update=[])
                        insts.insert(idx, nd)
                        idx += 1
                idx += 1

    for fn in nc.m.functions:
        for bb in fn.blocks:
            for inst in bb.instructions:
                si = inst.sync_info
                if si is None or not si.on_wait or len(si.on_wait) < 2:
                    continue
                if type(inst).__name__ not in _SELF_WAIT_OPCODES:
                    continue
                eng = getattr(inst.engine, "name", str(inst.engine))
                prefix = None
                for k, v in _ENGINE_SEM_PREFIX.items():
                    if k in str(eng).upper():
                        prefix = v
                        break
                if prefix is None:
                    continue
                waits = list(si.on_wait)
                keep = [w for w in waits if not w.ant_name.startswith(prefix)]
                if len(keep) != len(waits) and keep:
                    inst.sync_info = mybir.SyncInfo(
                        on_wait=keep, on_update=si.on_update
                    )


def _finalize(results, pred_pos, edges) -> np.float32:
    s_all = 0.0
    for i, r in enumerate(results):
        acc = r["acc"].astype(np.float64)
        diag = acc[:, :SLOTS].sum()  # intra-tile blocks: each ordered pair once
        inter = acc[:, SLOTS:].sum()  # forward windows: each unordered pair once
        s_all += diag + 2.0 * inter
    corr = _edge_correction(pred_pos, edges)
    return np.float32(-s_all + 2.0 * corr)


def kernel(pred_pos: np.ndarray, edges: np.ndarray) -> np.ndarray:
    from concourse.bass_utils import run_bass_kernel_spmd

    in_maps, meta = _host_prep(pred_pos)
    nc = _build_program(meta)
    trace = os.environ.get("KERNEL_TRACE", "0") == "1"
    res = run_bass_kernel_spmd(
        nc,
        in_maps,
        core_ids=list(range(NCORES)),
        trace=trace,
    )
    LAST_RESULT["exec_time_ns"] = res.exec_time_ns
    LAST_RESULT["trace"] = res.instructions_and_trace
    LAST_RESULT["meta"] = meta

    return _finalize(res.results, pred_pos, edges)


# revision 19
# speedup vs baseline: 6.9810x; 1.2845x over previous
"""Trainium2 Bass kernel for the pairwise-distance masked log-sum loss.

Reference math (N=8192 points, E=49152 edges):
    dist[i,j] = |p_i - p_j|^2 + 1e-8
    mask      = (dist <= 0.25), edges (both directions) and diagonal zeroed
    loss      = sum(-log(dist) * mask)

Device strategy (8 NeuronCores, SPMD):
  * Points are sorted by x on the host.  Only pairs with |dx| < 0.5 can be
    inside the threshold, so each 128-row tile only needs its own blocks
    (intra-tile pairs) plus a forward window of sorted columns reaching
    x_max(tile) + 0.5.  Pairs outside the window contribute exactly 0.
    Forward windows count each inter-tile pair once; the host doubles that
    partial sum (dist is symmetric).  The 64 row tiles are dealt to the 8
    cores by snake order of window size, so every core runs an identical
    program on identically-shaped inputs (windows padded with far-away
    dummy points, which fall outside the threshold and contribute 0).
  * dist[i,j] = w_i . u_j with K=16 split-precision channels evaluated on
    the TensorE in float32r.  f32r is fp32 rounded to 11 explicit mantissa
    bits (measured round-to-nearest on hardware) at full bf16 PE rate; the
    host splits each coordinate c = ch + cl and |p|^2 = sqh + sql with
    rn11 so every channel is exactly representable and the product sum
    reconstructs |p_i - p_j|^2 + |p|^2-rounding (~1e-6) -- fp32-grade.
  * The diagonal 128x128 block has +10*I accumulated onto it by a second
    matmul (identity lhsT) so ln never sees the dist ~ 1e-8 diagonal.
  * ScalarE computes y = ln(dist) PSUM->SBUF (bf16), VectorE computes
    sum(y * (y <= ln .25)) per partition with the fused
    scalar_tensor_tensor accumulate.
  * Host: loss = -(S_diag + 2*S_windows) + 2*sum(ln dist) over the unique
    non-self edge pairs inside the threshold (the reference masks those
    out, the device sum includes them).
"""

import os

import numpy as np

N = 8192
NCORES = 8
ROW_TILE = 128
TILES = N // ROW_TILE  # 64
SLOTS = TILES // NCORES  # 8 row-tiles per core
KCH = 16  # split-precision channels
COL_CHUNK = 512  # one PSUM bank per matmul
GROUP_COLS = 2048  # ACT/DVE group = 4 PSUM banks
EPS = 1e-8
THR2 = 0.25
XWIN = 0.5
LN_THR = float(np.log(0.25))

USE_FP32R = os.environ.get("KERNEL_FP32R", "1") == "1"
ACC_SLOTS = 64

LAST_RESULT = {}


def _rn11(v: np.ndarray) -> np.ndarray:
    """Round f32/f64 values to 11 explicit mantissa bits (round-to-nearest)
    -- the measured float32r grid."""
    v64 = np.asarray(v, dtype=np.float64)
    m, e = np.frexp(v64)
    q = np.ldexp(np.round(np.ldexp(m, 12)) / (1 << 12), e)
    return q.astype(np.float32)


def _build_channels(pts: np.ndarray):
    """w [16, n] and u [16, n] channel vectors, f32r-grid values, such that
    sum_k w[k,i]*u[k,j] ~= |p_i - p_j|^2 (+eps folded into sql)."""
    c = np.asarray(pts, dtype=np.float32)
    ch = _rn11(c)
    cl = _rn11(c.astype(np.float64) - ch)
    rep = ch.astype(np.float64) + cl  # represented points
    sq = (rep * rep).sum(axis=1)  # f64, exact-ish
    sqh = _rn11(sq)
    sql = _rn11(sq - sqh)

    n = c.shape[0]
    w = np.empty((KCH, n), np.float32)
    u = np.empty((KCH, n), np.float32)
    for a in range(3):
        w[4 * a + 0] = -2.0 * ch[:, a]
        u[4 * a + 0] = ch[:, a]
        w[4 * a + 1] = -2.0 * ch[:, a]
        u[4 * a + 1] = cl[:, a]
        w[4 * a + 2] = -2.0 * cl[:, a]
        u[4 * a + 2] = ch[:, a]
        w[4 * a + 3] = -2.0 * cl[:, a]
        u[4 * a + 3] = cl[:, a]
    w[12] = sqh
    u[12] = 1.0
    w[13] = sql
    u[13] = 1.0
    w[14] = 1.0
    u[14] = sqh
    w[15] = 1.0
    u[15] = sql
    return w, u


BANDS = 8


def _host_prep(pred_pos: np.ndarray):
    """Two-level sort (x-bands, y within band), per-tile geometric windows,
    snake balance; build per-core in_maps and program meta.

    Each row tile's window = [its own 128 columns] + every forward column
    that could be within the 0.5 threshold: same/later bands whose x-range
    is reachable, restricted to the tile's y-range +- 0.5.  Every unordered
    off-diagonal pair inside the threshold appears exactly once (own-tile
    lower triangle and diagonal are pushed out of the mask by the +10
    lower-tri matmul); the host doubles the device sum."""
    p = np.asarray(pred_pos, dtype=np.float32)
    per = N // BANDS
    xi = np.argsort(p[:, 0], kind="stable")
    psx = p[xi]
    order_parts = []
    band_x = []
    for b in range(BANDS):
        seg = np.arange(b * per, (b + 1) * per)
        band_x.append(
            (float(psx[seg, 0].min()), float(psx[seg, 0].max()))
        )
        yi = np.argsort(psx[seg, 1], kind="stable")
        order_parts.append(seg[yi])
    order = np.concatenate(order_parts)
    ps = psx[order]
    ys_band = [ps[b * per : (b + 1) * per, 1].astype(np.float64) for b in range(BANDS)]

    w, u = _build_channels(ps)

    CUSH = 1e-3
    tile_ranges = []  # per tile: list of (lo, hi) global column ranges
    for t in range(TILES):
        t0, t1 = t * ROW_TILE, (t + 1) * ROW_TILE
        b = t0 // per
        ya = float(ps[t0:t1, 1].min())
        yb = float(ps[t0:t1, 1].max())
        ranges = [(t0, t1)]  # own tile first (lower-tri masked on device)
        for b2 in range(b, BANDS):
            if band_x[b2][0] - band_x[b][1] >= XWIN - CUSH:
                break
            lo = int(np.searchsorted(ys_band[b2], ya - XWIN - CUSH))
            hi = int(np.searchsorted(ys_band[b2], yb + XWIN + CUSH, side="right"))
            lo += b2 * per
            hi += b2 * per
            if b2 == b:
                lo = max(lo, t1)
            if hi > lo:
                ranges.append((lo, hi))
        tile_ranges.append(ranges)

    widths = [sum(hi - lo for lo, hi in r) for r in tile_ranges]

    # snake-deal tiles (desc width) to cores; slot s width = max in band
    rank = sorted(range(TILES), key=lambda t: -widths[t])
    assign = [[None] * SLOTS for _ in range(NCORES)]
    for s in range(SLOTS):
        band = rank[s * NCORES : (s + 1) * NCORES]
        cores = range(NCORES) if s % 2 == 0 else range(NCORES - 1, -1, -1)
        for t, c in zip(band, cores):
            assign[c][s] = t
    slot_w = []
    for s in range(SLOTS):
        wmax = max(widths[assign[c][s]] for c in range(NCORES))
        slot_w.append(int(np.ceil(wmax / ROW_TILE)) * ROW_TILE)

    # dummy far-away point channels (outside any threshold window)
    _, ud = _build_channels(np.array([[1000.0, 0.0, 0.0]], np.float32))

    # per-core packed input [KCH, SLOTS*128 (rowsW) | windows]
    width = SLOTS * ROW_TILE + sum(slot_w)
    in_maps = []
    lmask_pad = np.zeros((128, COL_CHUNK), np.float32)
    lmask_pad[:, :128] = np.tril(np.full((128, 128), 10.0, np.float32))
    idid = np.concatenate(
        [np.eye(128, dtype=np.float32), lmask_pad], axis=1
    ).astype(np.float32)
    for c in range(NCORES):
        inp = np.empty((KCH, width), np.float32)
        for s in range(SLOTS):
            t = assign[c][s]
            r0 = t * ROW_TILE
            inp[:, s * ROW_TILE : (s + 1) * ROW_TILE] = w[:, r0 : r0 + ROW_TILE]
        off = SLOTS * ROW_TILE
        for s in range(SLOTS):
            t = assign[c][s]
            o = off
            for lo, hi in tile_ranges[t]:
                inp[:, o : o + hi - lo] = u[:, lo:hi]
                o += hi - lo
            if o < off + slot_w[s]:
                inp[:, o : off + slot_w[s]] = ud
            off += slot_w[s]
        in_maps.append({"inp": np.ascontiguousarray(inp), "idid": idid})

    meta = {"slot_w": slot_w, "width": width}
    return in_maps, meta


def _edge_correction(pred_pos: np.ndarray, edges: np.ndarray) -> float:
    """sum of ln(dist) over unique unordered non-self edge pairs inside the
    threshold (each such pair appears exactly twice in the device sum)."""
    p = np.asarray(pred_pos, dtype=np.float32)
    e = np.asarray(edges, dtype=np.int64)
    e = e[e[:, 0] != e[:, 1]]
    e = np.sort(e, axis=1)
    e = np.unique(e, axis=0)
    d = p[e[:, 0]] - p[e[:, 1]]
    dist = (d * d).sum(axis=1, dtype=np.float32) + np.float32(EPS)
    m = dist <= np.float32(THR2)
    return float(np.log(dist[m].astype(np.float64)).sum())


def _build_program(meta):
    import concourse.bass as bass
    import concourse.tile as tile
    from concourse import mybir
    from contextlib import ExitStack

    f32 = mybir.dt.float32
    bf16 = mybir.dt.bfloat16
    in_dt = mybir.dt.float32r if USE_FP32R else f32

    slot_w = meta["slot_w"]
    width = meta["width"]

    nc = bass.Bass("TRN2", target_bir_lowering=False, debug=False, num_devices=NCORES)
    inp_d = nc.dram_tensor("inp", [KCH, width], in_dt, kind="ExternalInput").ap()
    idid_d = nc.dram_tensor(
        "idid", [128, 128 + COL_CHUNK], in_dt, kind="ExternalInput"
    ).ap()
    acc_d = nc.dram_tensor("acc", [128, ACC_SLOTS], f32, kind="ExternalOutput").ap()

    # how many ACT/DVE groups in total (for y-buffer count: no slot reuse)
    n_groups = sum((wl + GROUP_COLS - 1) // GROUP_COLS for wl in slot_w)
    assert n_groups <= ACC_SLOTS

    with tile.TileContext(nc) as tc, ExitStack() as ctx:
        singles = ctx.enter_context(tc.tile_pool(name="singles", bufs=1))
        psums = ctx.enter_context(tc.tile_pool(name="psums", bufs=2, space="PSUM"))
        ys = ctx.enter_context(tc.tile_pool(name="ys", bufs=n_groups))
        scraps = ctx.enter_context(tc.tile_pool(name="scraps", bufs=2))

        inp_s = singles.tile([KCH, width], in_dt)
        nc.sync.dma_start(out=inp_s, in_=inp_d)
        idid_s = singles.tile([128, 128 + COL_CHUNK], in_dt)
        nc.sync.dma_start(out=idid_s, in_=idid_d)
        acc_s = singles.tile([128, ACC_SLOTS], f32)

        id1 = idid_s[:, :128]
        lmask = idid_s[:, 128:]

        def reduce_group(psum_t, cols, acc_idx):
            y_t = ys.tile([128, GROUP_COLS], bf16, tag="y")
            nc.scalar.activation(
                out=y_t[:, :cols],
                in_=psum_t[:, :cols],
                func=mybir.ActivationFunctionType.Ln,
            )
            scrap_t = scraps.tile([128, GROUP_COLS], bf16, tag="scrap")
            nc.vector.scalar_tensor_tensor(
                out=scrap_t[:, :cols],
                in0=y_t[:, :cols],
                scalar=LN_THR,
                in1=y_t[:, :cols],
                op0=mybir.AluOpType.is_le,
                op1=mybir.AluOpType.mult,
                accum_out=acc_s[:, acc_idx : acc_idx + 1],
            )

        # Each slot's window = [own 128 cols | forward cols]; the own-tile
        # lower triangle and diagonal are pushed out of the ln-threshold
        # mask by accumulating +10*tril onto psum cols [0, 128).
        acc_idx = 0
        win_off = SLOTS * ROW_TILE
        for s in range(SLOTS):
            lhsT = inp_s[:, s * ROW_TILE : (s + 1) * ROW_TILE]
            wl = slot_w[s]
            done = 0
            while done < wl:
                cols = min(GROUP_COLS, wl - done)
                psum_t = psums.tile([128, GROUP_COLS], f32, tag="ps")
                for k0 in range(0, cols, COL_CHUNK):
                    kw = min(COL_CHUNK, cols - k0)
                    c0 = win_off + done + k0
                    first = done == 0 and k0 == 0
                    nc.tensor.matmul(
                        out=psum_t[:, k0 : k0 + kw],
                        lhsT=lhsT,
                        rhs=inp_s[:, c0 : c0 + kw],
                        start=True,
                        stop=not first,
                    )
                    if first:
                        # +10 * tril on the own-tile block; padded to the
                        # full chunk so the accumulation group APs match
                        nc.tensor.matmul(
                            out=psum_t[:, k0 : k0 + kw],
                            lhsT=id1,
                            rhs=lmask[:, :kw],
                            start=False,
                            stop=True,
                        )
                reduce_group(psum_t, cols, acc_idx)
                acc_idx += 1
                done += cols
            win_off += wl
        assert acc_idx <= ACC_SLOTS

        nc.sync.dma_start(out=acc_d, in_=acc_s)

    _strip_self_waits(nc, mybir)
    return nc


_SELF_WAIT_OPCODES = {
    "InstMatmult",
    "InstTensorScalarPtr",
    "InstActivation",
    "InstTensorTensor",
    "InstTensorReduce",
    "InstTensorCopy",
    "InstMemset",
}
_ENGINE_SEM_PREFIX = {
    "PE": "PE_",
    "ACT": "Activation_",
    "DVE": "DVE_",
    "POOL": "Pool_",
    "SP": "SP_",
}


def _strip_self_waits(nc, mybir):
    """Walrus caps sync-wait commands per instruction (1 for PE/DVE compute
    structs).  Tile conservatively emits same-engine self-waits alongside
    the real cross-engine waits; the engines are in-order so self-waits are
    vacuous -- drop them.  Multi-wait Drains (kernel tail) are split into
    chains of single-wait drains (sequentially equivalent)."""
    split_id = 0
    for fn in nc.m.functions:
        for bb in fn.blocks:
            idx = 0
            insts = bb.instructions
            while idx < len(insts):
                inst = insts[idx]
                si = inst.sync_info
                if (
                    type(inst).__name__ == "InstDrain"
                    and si is not None
                    and si.on_wait
                    and len(si.on_wait) > 1
                ):
                    waits = list(si.on_wait)
                    inst.sync_info = mybir.SyncInfo(
                        on_wait=[waits[-1]], on_update=si.on_update
                    )
                    for w in waits[:-1]:
                        nd = mybir.InstDrain(
                            name=f"I-drainsplit-{split_id}",
                            ins=[],
                            outs=[],
                            bass_is_fusable=False,
                        )
                        split_id += 1
                        nd.engine = inst.engine
                        nd.sync_info = mybir.SyncInfo(on_wait=[w], on_update=[])
                        insts.insert(idx, nd)
                        idx += 1
                idx += 1

    for fn in nc.m.functions:
        for bb in fn.blocks:
            for inst in bb.instructions:
                si = inst.sync_info
                if si is None or not si.on_wait or len(si.on_wait) < 2:
                    continue
                if type(inst).__name__ not in _SELF_WAIT_OPCODES:
                    continue
                eng = getattr(inst.engine, "name", str(inst.engine))
                prefix = None
                for k, v in _ENGINE_SEM_PREFIX.items():
                    if k in str(eng).upper():
                        prefix = v
                        break
                if prefix is None:
                    continue
                waits = list(si.on_wait)
                keep = [w for w in waits if not w.ant_name.startswith(prefix)]
                if len(keep) != len(waits) and keep:
                    inst.sync_info = mybir.SyncInfo(
                        on_wait=keep, on_update=si.on_update
                    )


def _finalize(results, pred_pos, edges) -> np.float32:
    # every unordered off-diagonal pair inside the threshold appears exactly
    # once in the device sum -> double it; edge pairs likewise.
    s_all = 0.0
    for r in results:
        s_all += r["acc"].astype(np.float64).sum()
    corr = _edge_correction(pred_pos, edges)
    return np.float32(-2.0 * s_all + 2.0 * corr)


def kernel(pred_pos: np.ndarray, edges: np.ndarray) -> np.ndarray:
    from concourse.bass_utils import run_bass_kernel_spmd

    in_maps, meta = _host_prep(pred_pos)
    nc = _build_program(meta)
    trace = os.environ.get("KERNEL_TRACE", "0") == "1"
    res = run_bass_kernel_spmd(
        nc,
        in_maps,
        core_ids=list(range(NCORES)),
        trace=trace,
    )
    LAST_RESULT["exec_time_ns"] = res.exec_time_ns
    LAST_RESULT["trace"] = res.instructions_and_trace
    LAST_RESULT["meta"] = meta

    return _finalize(res.results, pred_pos, edges)


# revision 22
# speedup vs baseline: 7.7199x; 1.1058x over previous
"""Trainium2 Bass kernel for the pairwise-distance masked log-sum loss.

Reference math (N=8192 points, E=49152 edges):
    dist[i,j] = |p_i - p_j|^2 + 1e-8
    mask      = (dist <= 0.25), edges (both directions) and diagonal zeroed
    loss      = sum(-log(dist) * mask)

Device strategy (8 NeuronCores, SPMD):
  * Points are sorted by x on the host.  Only pairs with |dx| < 0.5 can be
    inside the threshold, so each 128-row tile only needs its own blocks
    (intra-tile pairs) plus a forward window of sorted columns reaching
    x_max(tile) + 0.5.  Pairs outside the window contribute exactly 0.
    Forward windows count each inter-tile pair once; the host doubles that
    partial sum (dist is symmetric).  The 64 row tiles are dealt to the 8
    cores by snake order of window size, so every core runs an identical
    program on identically-shaped inputs (windows padded with far-away
    dummy points, which fall outside the threshold and contribute 0).
  * dist[i,j] = w_i . u_j with K=16 split-precision channels evaluated on
    the TensorE in float32r.  f32r is fp32 rounded to 11 explicit mantissa
    bits (measured round-to-nearest on hardware) at full bf16 PE rate; the
    host splits each coordinate c = ch + cl and |p|^2 = sqh + sql with
    rn11 so every channel is exactly representable and the product sum
    reconstructs |p_i - p_j|^2 + |p|^2-rounding (~1e-6) -- fp32-grade.
  * The diagonal 128x128 block has +10*I accumulated onto it by a second
    matmul (identity lhsT) so ln never sees the dist ~ 1e-8 diagonal.
  * ScalarE computes y = ln(dist) PSUM->SBUF (bf16), VectorE computes
    sum(y * (y <= ln .25)) per partition with the fused
    scalar_tensor_tensor accumulate.
  * Host: loss = -(S_diag + 2*S_windows) + 2*sum(ln dist) over the unique
    non-self edge pairs inside the threshold (the reference masks those
    out, the device sum includes them).
"""

import os

import numpy as np

N = 8192
NCORES = 8
ROW_TILE = 128
TILES = N // ROW_TILE  # 64
SLOTS = TILES // NCORES  # 8 row-tiles per core
KCH = 16  # split-precision channels
COL_CHUNK = 512  # one PSUM bank per matmul
GROUP_COLS = 2048  # ACT/DVE group = 4 PSUM banks
EPS = 1e-8
THR2 = 0.25
XWIN = 0.5
LN_THR = float(np.log(0.25))

USE_FP32R = os.environ.get("KERNEL_FP32R", "1") == "1"
ACC_SLOTS = 64

LAST_RESULT = {}


def _rn11(v: np.ndarray) -> np.ndarray:
    """Round f32/f64 values to 11 explicit mantissa bits (round-to-nearest)
    -- the measured float32r grid."""
    v64 = np.asarray(v, dtype=np.float64)
    m, e = np.frexp(v64)
    q = np.ldexp(np.round(np.ldexp(m, 12)) / (1 << 12), e)
    return q.astype(np.float32)


def _build_channels(pts: np.ndarray):
    """w [16, n] and u [16, n] channel vectors, f32r-grid values, such that
    sum_k w[k,i]*u[k,j] ~= |p_i - p_j|^2 (+eps folded into sql)."""
    c = np.asarray(pts, dtype=np.float32)
    ch = _rn11(c)
    cl = _rn11(c.astype(np.float64) - ch)
    rep = ch.astype(np.float64) + cl  # represented points
    sq = (rep * rep).sum(axis=1)  # f64, exact-ish
    sqh = _rn11(sq)
    sql = _rn11(sq - sqh)

    n = c.shape[0]
    w = np.empty((KCH, n), np.float32)
    u = np.empty((KCH, n), np.float32)
    for a in range(3):
        w[4 * a + 0] = -2.0 * ch[:, a]
        u[4 * a + 0] = ch[:, a]
        w[4 * a + 1] = -2.0 * ch[:, a]
        u[4 * a + 1] = cl[:, a]
        w[4 * a + 2] = -2.0 * cl[:, a]
        u[4 * a + 2] = ch[:, a]
        w[4 * a + 3] = -2.0 * cl[:, a]
        u[4 * a + 3] = cl[:, a]
    w[12] = sqh
    u[12] = 1.0
    w[13] = sql
    u[13] = 1.0
    w[14] = 1.0
    u[14] = sqh
    w[15] = 1.0
    u[15] = sql
    return w, u


BANDS = 8


def _host_prep(pred_pos: np.ndarray):
    """Two-level sort (x-bands, y within band), per-tile geometric windows,
    snake balance; build per-core in_maps and program meta.

    Each row tile's window = [its own 128 columns] + every forward column
    that could be within the 0.5 threshold: same/later bands whose x-range
    is reachable, restricted to the tile's y-range +- 0.5.  Every unordered
    off-diagonal pair inside the threshold appears exactly once (own-tile
    lower triangle and diagonal are pushed out of the mask by the +10
    lower-tri matmul); the host doubles the device sum."""
    p = np.asarray(pred_pos, dtype=np.float32)
    per = N // BANDS
    xi = np.argsort(p[:, 0], kind="stable")
    psx = p[xi]
    order_parts = []
    band_x = []
    for b in range(BANDS):
        seg = np.arange(b * per, (b + 1) * per)
        band_x.append(
            (float(psx[seg, 0].min()), float(psx[seg, 0].max()))
        )
        yi = np.argsort(psx[seg, 1], kind="stable")
        order_parts.append(seg[yi])
    order = np.concatenate(order_parts)
    ps = psx[order]
    ys_band = [ps[b * per : (b + 1) * per, 1].astype(np.float64) for b in range(BANDS)]

    w, u = _build_channels(ps)

    CUSH = 1e-3
    tile_ranges = []  # per tile: list of (lo, hi) global column ranges
    for t in range(TILES):
        t0, t1 = t * ROW_TILE, (t + 1) * ROW_TILE
        b = t0 // per
        ya = float(ps[t0:t1, 1].min())
        yb = float(ps[t0:t1, 1].max())
        ranges = [(t0, t1)]  # own tile first (lower-tri masked on device)
        for b2 in range(b, BANDS):
            if band_x[b2][0] - band_x[b][1] >= XWIN - CUSH:
                break
            lo = int(np.searchsorted(ys_band[b2], ya - XWIN - CUSH))
            hi = int(np.searchsorted(ys_band[b2], yb + XWIN + CUSH, side="right"))
            lo += b2 * per
            hi += b2 * per
            if b2 == b:
                lo = max(lo, t1)
            if hi > lo:
                ranges.append((lo, hi))
        tile_ranges.append(ranges)

    widths = [sum(hi - lo for lo, hi in r) for r in tile_ranges]

    # snake-deal tiles (desc width) to cores; slot s width = max in band
    rank = sorted(range(TILES), key=lambda t: -widths[t])
    assign = [[None] * SLOTS for _ in range(NCORES)]
    for s in range(SLOTS):
        band = rank[s * NCORES : (s + 1) * NCORES]
        cores = range(NCORES) if s % 2 == 0 else range(NCORES - 1, -1, -1)
        for t, c in zip(band, cores):
            assign[c][s] = t
    slot_w = []
    for s in range(SLOTS):
        wmax = max(widths[assign[c][s]] for c in range(NCORES))
        slot_w.append(int(np.ceil(wmax / ROW_TILE)) * ROW_TILE)

    # dummy far-away point channels (outside any threshold window)
    _, ud = _build_channels(np.array([[1000.0, 0.0, 0.0]], np.float32))

    # per-core packed input [KCH, SLOTS*128 (rowsW) | windows]
    width = SLOTS * ROW_TILE + sum(slot_w)
    in_maps = []
    lmask_pad = np.zeros((128, COL_CHUNK), np.float32)
    lmask_pad[:, :128] = np.tril(np.full((128, 128), 10.0, np.float32))
    idid = np.concatenate(
        [np.eye(128, dtype=np.float32), lmask_pad], axis=1
    ).astype(np.float32)
    for c in range(NCORES):
        inp = np.empty((KCH, width), np.float32)
        for s in range(SLOTS):
            t = assign[c][s]
            r0 = t * ROW_TILE
            inp[:, s * ROW_TILE : (s + 1) * ROW_TILE] = w[:, r0 : r0 + ROW_TILE]
        off = SLOTS * ROW_TILE
        for s in range(SLOTS):
            t = assign[c][s]
            o = off
            for lo, hi in tile_ranges[t]:
                inp[:, o : o + hi - lo] = u[:, lo:hi]
                o += hi - lo
            if o < off + slot_w[s]:
                inp[:, o : off + slot_w[s]] = ud
            off += slot_w[s]
        in_maps.append({"inp": np.ascontiguousarray(inp), "idid": idid})

    meta = {"slot_w": slot_w, "width": width}
    return in_maps, meta


def _edge_correction(pred_pos: np.ndarray, edges: np.ndarray) -> float:
    """sum of ln(dist) over unique unordered non-self edge pairs inside the
    threshold (each such pair appears exactly twice in the device sum)."""
    p = np.asarray(pred_pos, dtype=np.float32)
    e = np.asarray(edges, dtype=np.int64)
    e = e[e[:, 0] != e[:, 1]]
    e = np.sort(e, axis=1)
    e = np.unique(e, axis=0)
    d = p[e[:, 0]] - p[e[:, 1]]
    dist = (d * d).sum(axis=1, dtype=np.float32) + np.float32(EPS)
    m = dist <= np.float32(THR2)
    return float(np.log(dist[m].astype(np.float64)).sum())


def _build_program(meta):
    import concourse.bass as bass
    import concourse.tile as tile
    from concourse import mybir
    from contextlib import ExitStack

    f32 = mybir.dt.float32
    bf16 = mybir.dt.bfloat16
    in_dt = mybir.dt.float32r if USE_FP32R else f32

    slot_w = meta["slot_w"]
    width = meta["width"]

    nc = bass.Bass("TRN2", target_bir_lowering=False, debug=False, num_devices=NCORES)
    inp_d = nc.dram_tensor("inp", [KCH, width], in_dt, kind="ExternalInput").ap()
    idid_d = nc.dram_tensor(
        "idid", [128, 128 + COL_CHUNK], in_dt, kind="ExternalInput"
    ).ap()
    acc_d = nc.dram_tensor("acc", [128, ACC_SLOTS], f32, kind="ExternalOutput").ap()

    # how many ACT/DVE groups in total (for y-buffer count: no slot reuse)
    n_groups = sum((wl + GROUP_COLS - 1) // GROUP_COLS for wl in slot_w)
    assert n_groups <= ACC_SLOTS

    with tile.TileContext(nc) as tc, ExitStack() as ctx:
        singles = ctx.enter_context(tc.tile_pool(name="singles", bufs=1))
        psums = ctx.enter_context(tc.tile_pool(name="psums", bufs=2, space="PSUM"))
        ys = ctx.enter_context(tc.tile_pool(name="ys", bufs=n_groups))
        scraps = ctx.enter_context(tc.tile_pool(name="scraps", bufs=2))

        # split the input DMA per slot so PE starts as soon as slot 0's
        # window has landed and later windows stream in under compute
        inp_s = singles.tile([KCH, width], in_dt)
        idid_s = singles.tile([128, 128 + COL_CHUNK], in_dt)
        nc.sync.dma_start(out=idid_s, in_=idid_d)
        cut = SLOTS * ROW_TILE + slot_w[0]
        nc.sync.dma_start(out=inp_s[:, :cut], in_=inp_d[:, :cut])
        for s in range(1, SLOTS):
            nc.sync.dma_start(
                out=inp_s[:, cut : cut + slot_w[s]],
                in_=inp_d[:, cut : cut + slot_w[s]],
            )
            cut += slot_w[s]
        acc_s = singles.tile([128, ACC_SLOTS], f32)

        id1 = idid_s[:, :128]
        lmask = idid_s[:, 128:]

        def reduce_group(psum_t, cols, acc_idx):
            y_t = ys.tile([128, GROUP_COLS], bf16, tag="y")
            nc.scalar.activation(
                out=y_t[:, :cols],
                in_=psum_t[:, :cols],
                func=mybir.ActivationFunctionType.Ln,
            )
            scrap_t = scraps.tile([128, GROUP_COLS], bf16, tag="scrap")
            nc.vector.scalar_tensor_tensor(
                out=scrap_t[:, :cols],
                in0=y_t[:, :cols],
                scalar=LN_THR,
                in1=y_t[:, :cols],
                op0=mybir.AluOpType.is_le,
                op1=mybir.AluOpType.mult,
                accum_out=acc_s[:, acc_idx : acc_idx + 1],
            )

        # Each slot's window = [own 128 cols | forward cols]; the own-tile
        # lower triangle and diagonal are pushed out of the ln-threshold
        # mask by accumulating +10*tril onto psum cols [0, 128).
        acc_idx = 0
        win_off = SLOTS * ROW_TILE
        for s in range(SLOTS):
            lhsT = inp_s[:, s * ROW_TILE : (s + 1) * ROW_TILE]
            wl = slot_w[s]
            done = 0
            while done < wl:
                cols = min(GROUP_COLS, wl - done)
                psum_t = psums.tile([128, GROUP_COLS], f32, tag="ps")
                for k0 in range(0, cols, COL_CHUNK):
                    kw = min(COL_CHUNK, cols - k0)
                    c0 = win_off + done + k0
                    first = done == 0 and k0 == 0
                    if first:
                        # +10 * tril on the own-tile block, issued first
                        # (start=True) so it absorbs the psum-reuse wait
                        # and the dist matmul carries only the DMA wait
                        # (walrus allows one sync wait per PE instruction).
                        # Padded to the chunk so the group APs match.
                        nc.tensor.matmul(
                            out=psum_t[:, k0 : k0 + kw],
                            lhsT=id1,
                            rhs=lmask[:, :kw],
                            start=True,
                            stop=False,
                        )
                    nc.tensor.matmul(
                        out=psum_t[:, k0 : k0 + kw],
                        lhsT=lhsT,
                        rhs=inp_s[:, c0 : c0 + kw],
                        start=not first,
                        stop=True,
                    )
                reduce_group(psum_t, cols, acc_idx)
                acc_idx += 1
                done += cols
            win_off += wl
        assert acc_idx <= ACC_SLOTS

        nc.sync.dma_start(out=acc_d, in_=acc_s)

    _strip_self_waits(nc, mybir)
    return nc


_SELF_WAIT_OPCODES = {
    "InstMatmult",
    "InstTensorScalarPtr",
    "InstActivation",
    "InstTensorTensor",
    "InstTensorReduce",
    "InstTensorCopy",
    "InstMemset",
}
_ENGINE_SEM_PREFIX = {
    "PE": "PE_",
    "ACT": "Activation_",
    "DVE": "DVE_",
    "POOL": "Pool_",
    "SP": "SP_",
}


def _strip_self_waits(nc, mybir):
    """Walrus caps sync-wait commands per instruction (1 for PE/DVE compute
    structs).  Tile conservatively emits same-engine self-waits alongside
    the real cross-engine waits; the engines are in-order so self-waits are
    vacuous -- drop them.  Multi-wait Drains (kernel tail) are split into
    chains of single-wait drains (sequentially equivalent)."""
    split_id = 0
    for fn in nc.m.functions:
        for bb in fn.blocks:
            idx = 0
            insts = bb.instructions
            while idx < len(insts):
                inst = insts[idx]
                si = inst.sync_info
                if (
                    type(inst).__name__ == "InstDrain"
                    and si is not None
                    and si.on_wait
                    and len(si.on_wait) > 1
                ):
                    waits = list(si.on_wait)
                    inst.sync_info = mybir.SyncInfo(
                        on_wait=[waits[-1]], on_update=si.on_update
                    )
                    for w in waits[:-1]:
                        nd = mybir.InstDrain(
                            name=f"I-drainsplit-{split_id}",
                            ins=[],
                            outs=[],
                            bass_is_fusable=False,
                        )
                        split_id += 1
                        nd.engine = inst.engine
                        nd.sync_info = mybir.SyncInfo(on_wait=[w], on_update=[])
                        insts.insert(idx, nd)
                        idx += 1
                idx += 1

    for fn in nc.m.functions:
        for bb in fn.blocks:
            for inst in bb.instructions:
                si = inst.sync_info
                if si is None or not si.on_wait or len(si.on_wait) < 2:
                    continue
                if type(inst).__name__ == "InstDMACopy":
                    # cross-queue DMA-ordering waits are not data deps here
                    # (all SBUF regions involved are disjoint); keep the
                    # engine wait that carries the real dependency.
                    waits = list(si.on_wait)
                    keep = [
                        w
                        for w in waits
                        if not w.ant_name.startswith(("DMAHW", "DMASW"))
                    ]
                    if keep and len(keep) < len(waits):
                        inst.sync_info = mybir.SyncInfo(
                            on_wait=keep, on_update=si.on_update
                        )
                    continue
                if type(inst).__name__ not in _SELF_WAIT_OPCODES:
                    continue
                eng = getattr(inst.engine, "name", str(inst.engine))
                prefix = None
                for k, v in _ENGINE_SEM_PREFIX.items():
                    if k in str(eng).upper():
                        prefix = v
                        break
                if prefix is None:
                    continue
                waits = list(si.on_wait)
                keep = [w for w in waits if not w.ant_name.startswith(prefix)]
                if len(keep) != len(waits) and keep:
                    inst.sync_info = mybir.SyncInfo(
                        on_wait=keep, on_update=si.on_update
                    )


def _finalize(results, pred_pos, edges) -> np.float32:
    # every unordered off-diagonal pair inside the threshold appears exactly
    # once in the device sum -> double it; edge pairs likewise.
    s_all = 0.0
    for r in results:
        s_all += r["acc"].astype(np.float64).sum()
    corr = _edge_correction(pred_pos, edges)
    return np.float32(-2.0 * s_all + 2.0 * corr)


def kernel(pred_pos: np.ndarray, edges: np.ndarray) -> np.ndarray:
    from concourse.bass_utils import run_bass_kernel_spmd

    in_maps, meta = _host_prep(pred_pos)
    nc = _build_program(meta)
    trace = os.environ.get("KERNEL_TRACE", "0") == "1"
    res = run_bass_kernel_spmd(
        nc,
        in_maps,
        core_ids=list(range(NCORES)),
        trace=trace,
    )
    LAST_RESULT["exec_time_ns"] = res.exec_time_ns
    LAST_RESULT["trace"] = res.instructions_and_trace
    LAST_RESULT["meta"] = meta

    return _finalize(res.results, pred_pos, edges)


# revision 25
# speedup vs baseline: 8.1055x; 1.0500x over previous
"""Trainium2 Bass kernel for the pairwise-distance masked log-sum loss.

Reference math (N=8192 points, E=49152 edges):
    dist[i,j] = |p_i - p_j|^2 + 1e-8
    mask      = (dist <= 0.25), edges (both directions) and diagonal zeroed
    loss      = sum(-log(dist) * mask)

Device strategy (8 NeuronCores, SPMD):
  * Points are sorted by x on the host.  Only pairs with |dx| < 0.5 can be
    inside the threshold, so each 128-row tile only needs its own blocks
    (intra-tile pairs) plus a forward window of sorted columns reaching
    x_max(tile) + 0.5.  Pairs outside the window contribute exactly 0.
    Forward windows count each inter-tile pair once; the host doubles that
    partial sum (dist is symmetric).  The 64 row tiles are dealt to the 8
    cores by snake order of window size, so every core runs an identical
    program on identically-shaped inputs (windows padded with far-away
    dummy points, which fall outside the threshold and contribute 0).
  * dist[i,j] = w_i . u_j with K=16 split-precision channels evaluated on
    the TensorE in float32r.  f32r is fp32 rounded to 11 explicit mantissa
    bits (measured round-to-nearest on hardware) at full bf16 PE rate; the
    host splits each coordinate c = ch + cl and |p|^2 = sqh + sql with
    rn11 so every channel is exactly representable and the product sum
    reconstructs |p_i - p_j|^2 + |p|^2-rounding (~1e-6) -- fp32-grade.
  * The diagonal 128x128 block has +10*I accumulated onto it by a second
    matmul (identity lhsT) so ln never sees the dist ~ 1e-8 diagonal.
  * ScalarE computes y = ln(dist) PSUM->SBUF (bf16), VectorE computes
    sum(y * (y <= ln .25)) per partition with the fused
    scalar_tensor_tensor accumulate.
  * Host: loss = -(S_diag + 2*S_windows) + 2*sum(ln dist) over the unique
    non-self edge pairs inside the threshold (the reference masks those
    out, the device sum includes them).
"""

import os

import numpy as np

N = 8192
NCORES = 8
ROW_TILE = 128
TILES = N // ROW_TILE  # 64
SLOTS = TILES // NCORES  # 8 row-tiles per core
KCH = 16  # split-precision channels
COL_CHUNK = 512  # one PSUM bank per matmul
GROUP_COLS = 2048  # ACT/DVE group = 4 PSUM banks
EPS = 1e-8
THR2 = 0.25
XWIN = 0.5
LN_THR = float(np.log(0.25))
DELTA = 6e-6  # positivity cushion folded into the u-side |p|^2 split

USE_FP32R = os.environ.get("KERNEL_FP32R", "1") == "1"
ACC_SLOTS = 64

LAST_RESULT = {}


def _rn11(v: np.ndarray) -> np.ndarray:
    """Round f32/f64 values to 11 explicit mantissa bits (round-to-nearest)
    -- the measured float32r grid."""
    v64 = np.asarray(v, dtype=np.float64)
    m, e = np.frexp(v64)
    q = np.ldexp(np.round(np.ldexp(m, 12)) / (1 << 12), e)
    return q.astype(np.float32)


def _build_channels(pts: np.ndarray):
    """w [16, n] and u [16, n] channel vectors, f32r-grid values, such that
    sum_k w[k,i]*u[k,j] ~= |p_i - p_j|^2 (+eps folded into sql)."""
    c = np.asarray(pts, dtype=np.float32)
    ch = _rn11(c)
    cl = _rn11(c.astype(np.float64) - ch)
    rep = ch.astype(np.float64) + cl  # represented points
    sq = (rep * rep).sum(axis=1)  # f64, exact-ish
    sqh = _rn11(sq)
    sql = _rn11(sq - sqh)
    # u-side |p|^2 carries +DELTA so every pair distance (in particular the
    # diagonal, which is pure split-residual ~ +-2e-6) stays positive for
    # Ln; the diagonal is pushed out of the mask afterwards (ymask).
    squ = sq + DELTA
    squh = _rn11(squ)
    squl = _rn11(squ - squh)

    n = c.shape[0]
    w = np.empty((KCH, n), np.float32)
    u = np.empty((KCH, n), np.float32)
    for a in range(3):
        w[4 * a + 0] = -2.0 * ch[:, a]
        u[4 * a + 0] = ch[:, a]
        w[4 * a + 1] = -2.0 * ch[:, a]
        u[4 * a + 1] = cl[:, a]
        w[4 * a + 2] = -2.0 * cl[:, a]
        u[4 * a + 2] = ch[:, a]
        w[4 * a + 3] = -2.0 * cl[:, a]
        u[4 * a + 3] = cl[:, a]
    w[12] = sqh
    u[12] = 1.0
    w[13] = sql
    u[13] = 1.0
    w[14] = 1.0
    u[14] = squh
    w[15] = 1.0
    u[15] = squl
    return w, u


BANDS = 8


def _host_prep(pred_pos: np.ndarray):
    """Two-level sort (x-bands, y within band), per-tile geometric windows,
    snake balance; build per-core in_maps and program meta.

    Each row tile's window = [its own 128 columns] + every forward column
    that could be within the 0.5 threshold: same/later bands whose x-range
    is reachable, restricted to the tile's y-range +- 0.5.  Every unordered
    off-diagonal pair inside the threshold appears exactly once (own-tile
    lower triangle and diagonal are pushed out of the mask by the +10
    lower-tri matmul); the host doubles the device sum."""
    p = np.asarray(pred_pos, dtype=np.float32)
    per = N // BANDS
    xi = np.argsort(p[:, 0], kind="stable")
    psx = p[xi]
    order_parts = []
    band_x = []
    for b in range(BANDS):
        seg = np.arange(b * per, (b + 1) * per)
        band_x.append(
            (float(psx[seg, 0].min()), float(psx[seg, 0].max()))
        )
        yi = np.argsort(psx[seg, 1], kind="stable")
        order_parts.append(seg[yi])
    order = np.concatenate(order_parts)
    ps = psx[order]
    ys_band = [ps[b * per : (b + 1) * per, 1].astype(np.float64) for b in range(BANDS)]

    w, u = _build_channels(ps)

    CUSH = 1e-3
    tile_ranges = []  # per tile: list of (lo, hi) global column ranges
    for t in range(TILES):
        t0, t1 = t * ROW_TILE, (t + 1) * ROW_TILE
        b = t0 // per
        ya = float(ps[t0:t1, 1].min())
        yb = float(ps[t0:t1, 1].max())
        ranges = [(t0, t1)]  # own tile first (lower-tri masked on device)
        for b2 in range(b, BANDS):
            if band_x[b2][0] - band_x[b][1] >= XWIN - CUSH:
                break
            lo = int(np.searchsorted(ys_band[b2], ya - XWIN - CUSH))
            hi = int(np.searchsorted(ys_band[b2], yb + XWIN + CUSH, side="right"))
            lo += b2 * per
            hi += b2 * per
            if b2 == b:
                lo = max(lo, t1)
            if hi > lo:
                ranges.append((lo, hi))
        tile_ranges.append(ranges)

    widths = [sum(hi - lo for lo, hi in r) for r in tile_ranges]

    # snake-deal tiles (desc width) to cores; slot s width = max in band
    rank = sorted(range(TILES), key=lambda t: -widths[t])
    assign = [[None] * SLOTS for _ in range(NCORES)]
    for s in range(SLOTS):
        band = rank[s * NCORES : (s + 1) * NCORES]
        cores = range(NCORES) if s % 2 == 0 else range(NCORES - 1, -1, -1)
        for t, c in zip(band, cores):
            assign[c][s] = t
    slot_w = []
    for s in range(SLOTS):
        wmax = max(widths[assign[c][s]] for c in range(NCORES))
        slot_w.append(int(np.ceil(wmax / ROW_TILE)) * ROW_TILE)

    # process the smallest slot first (quick PE start while the rest of the
    # input streams in), then descending so the kernel tail is short
    perm = sorted(range(SLOTS), key=lambda s: slot_w[s])
    perm = [perm[0]] + sorted(perm[1:], key=lambda s: -slot_w[s])
    slot_w = [slot_w[s] for s in perm]
    assign = [[assign[c][perm[s]] for s in range(SLOTS)] for c in range(NCORES)]

    # dummy far-away point channels (outside any threshold window)
    _, ud = _build_channels(np.array([[1000.0, 0.0, 0.0]], np.float32))

    # per-core packed input [KCH, SLOTS*128 (rowsW) | windows]
    width = SLOTS * ROW_TILE + sum(slot_w)
    in_maps = []
    import ml_dtypes

    ymask = np.tril(np.full((128, 128), 50.0, np.float32)).astype(ml_dtypes.bfloat16)
    for c in range(NCORES):
        inp = np.empty((KCH, width), np.float32)
        for s in range(SLOTS):
            t = assign[c][s]
            r0 = t * ROW_TILE
            inp[:, s * ROW_TILE : (s + 1) * ROW_TILE] = w[:, r0 : r0 + ROW_TILE]
        off = SLOTS * ROW_TILE
        for s in range(SLOTS):
            t = assign[c][s]
            o = off
            for lo, hi in tile_ranges[t]:
                inp[:, o : o + hi - lo] = u[:, lo:hi]
                o += hi - lo
            if o < off + slot_w[s]:
                inp[:, o : off + slot_w[s]] = ud
            off += slot_w[s]
        in_maps.append({"inp": np.ascontiguousarray(inp), "ymask": ymask})

    meta = {"slot_w": slot_w, "width": width}
    return in_maps, meta


def _edge_correction(pred_pos: np.ndarray, edges: np.ndarray) -> float:
    """sum of ln(dist) over unique unordered non-self edge pairs inside the
    threshold (each such pair appears exactly twice in the device sum)."""
    p = np.asarray(pred_pos, dtype=np.float32)
    e = np.asarray(edges, dtype=np.int64)
    e = e[e[:, 0] != e[:, 1]]
    e = np.sort(e, axis=1)
    e = np.unique(e, axis=0)
    d = p[e[:, 0]] - p[e[:, 1]]
    dist = (d * d).sum(axis=1, dtype=np.float32) + np.float32(EPS)
    m = dist <= np.float32(THR2)
    return float(np.log(dist[m].astype(np.float64)).sum())


def _build_program(meta):
    import concourse.bass as bass
    import concourse.tile as tile
    from concourse import mybir
    from contextlib import ExitStack

    f32 = mybir.dt.float32
    bf16 = mybir.dt.bfloat16
    in_dt = mybir.dt.float32r if USE_FP32R else f32

    slot_w = meta["slot_w"]
    width = meta["width"]

    nc = bass.Bass("TRN2", target_bir_lowering=False, debug=False, num_devices=NCORES)
    inp_d = nc.dram_tensor("inp", [KCH, width], in_dt, kind="ExternalInput").ap()
    ymask_d = nc.dram_tensor("ymask", [128, 128], bf16, kind="ExternalInput").ap()
    acc_d = nc.dram_tensor("acc", [128, ACC_SLOTS], f32, kind="ExternalOutput").ap()

    # how many ACT/DVE groups in total (for y-buffer count: no slot reuse)
    n_groups = sum((wl + GROUP_COLS - 1) // GROUP_COLS for wl in slot_w)
    assert n_groups <= ACC_SLOTS

    with tile.TileContext(nc) as tc, ExitStack() as ctx:
        singles = ctx.enter_context(tc.tile_pool(name="singles", bufs=1))
        psums = ctx.enter_context(tc.tile_pool(name="psums", bufs=2, space="PSUM"))
        ys = ctx.enter_context(tc.tile_pool(name="ys", bufs=n_groups))
        scraps = ctx.enter_context(tc.tile_pool(name="scraps", bufs=2))

        # split the input DMA per slot so PE starts as soon as slot 0's
        # window has landed and later windows stream in under compute
        inp_s = singles.tile([KCH, width], in_dt)
        ymask_s = singles.tile([128, 128], bf16)
        nc.sync.dma_start(out=ymask_s, in_=ymask_d)
        cut = SLOTS * ROW_TILE
        nc.sync.dma_start(out=inp_s[:, :cut], in_=inp_d[:, :cut])
        for s in range(SLOTS):
            nc.sync.dma_start(
                out=inp_s[:, cut : cut + slot_w[s]],
                in_=inp_d[:, cut : cut + slot_w[s]],
            )
            cut += slot_w[s]
        acc_s = singles.tile([128, ACC_SLOTS], f32)

        def reduce_group(psum_t, cols, acc_idx, mask_own):
            y_t = ys.tile([128, GROUP_COLS], bf16, tag="y")
            nc.scalar.activation(
                out=y_t[:, :cols],
                in_=psum_t[:, :cols],
                func=mybir.ActivationFunctionType.Ln,
            )
            if mask_own:
                # +50 on the own-tile lower triangle and diagonal pushes
                # those y values far above the threshold -> excluded
                nc.vector.tensor_tensor(
                    out=y_t[:, :ROW_TILE],
                    in0=y_t[:, :ROW_TILE],
                    in1=ymask_s,
                    op=mybir.AluOpType.add,
                )
            scrap_t = scraps.tile([128, GROUP_COLS], bf16, tag="scrap")
            nc.vector.scalar_tensor_tensor(
                out=scrap_t[:, :cols],
                in0=y_t[:, :cols],
                scalar=LN_THR,
                in1=y_t[:, :cols],
                op0=mybir.AluOpType.is_le,
                op1=mybir.AluOpType.mult,
                accum_out=acc_s[:, acc_idx : acc_idx + 1],
            )

        # Each slot's window = [own 128 cols | forward cols]; the own-tile
        # lower triangle and diagonal are pushed out of the ln-threshold
        # mask by accumulating +10*tril onto psum cols [0, 128).
        acc_idx = 0
        win_off = SLOTS * ROW_TILE
        for s in range(SLOTS):
            lhsT = inp_s[:, s * ROW_TILE : (s + 1) * ROW_TILE]
            wl = slot_w[s]
            done = 0
            while done < wl:
                cols = min(GROUP_COLS, wl - done)
                psum_t = psums.tile([128, GROUP_COLS], f32, tag="ps")
                for k0 in range(0, cols, COL_CHUNK):
                    kw = min(COL_CHUNK, cols - k0)
                    c0 = win_off + done + k0
                    nc.tensor.matmul(
                        out=psum_t[:, k0 : k0 + kw],
                        lhsT=lhsT,
                        rhs=inp_s[:, c0 : c0 + kw],
                        start=True,
                        stop=True,
                    )
                reduce_group(psum_t, cols, acc_idx, done == 0)
                acc_idx += 1
                done += cols
            win_off += wl
        assert acc_idx <= ACC_SLOTS

        nc.sync.dma_start(out=acc_d, in_=acc_s)

    _strip_self_waits(nc, mybir)
    return nc


_SELF_WAIT_OPCODES = {
    "InstMatmult",
    "InstTensorScalarPtr",
    "InstActivation",
    "InstTensorTensor",
    "InstTensorReduce",
    "InstTensorCopy",
    "InstMemset",
}
_ENGINE_SEM_PREFIX = {
    "PE": "PE_",
    "ACT": "Activation_",
    "DVE": "DVE_",
    "POOL": "Pool_",
    "SP": "SP_",
}


def _strip_self_waits(nc, mybir):
    """Walrus caps sync-wait commands per instruction (1 for PE/DVE compute
    structs).  Two post-passes make every instruction single-wait:
      1. drop same-engine self-waits on compute ops (the engines execute
         in order, so an instruction can never run before its same-engine
         predecessors complete; DVE additionally drains between ops), and
         cross-queue DMA-ordering waits on DMACopy (all SBUF regions
         involved here are disjoint);
      2. hoist any remaining extra waits onto same-engine Drain
         instructions inserted just before (an in-order queue enforces
         them for everything after)."""
    for fn in nc.m.functions:
        for bb in fn.blocks:
            for inst in bb.instructions:
                si = inst.sync_info
                if si is None or not si.on_wait or len(si.on_wait) < 2:
                    continue
                tname = type(inst).__name__
                waits = list(si.on_wait)
                if tname == "InstDMACopy":
                    keep = [
                        w
                        for w in waits
                        if not w.ant_name.startswith(("DMAHW", "DMASW"))
                    ]
                elif tname in _SELF_WAIT_OPCODES:
                    eng = getattr(inst.engine, "name", str(inst.engine))
                    prefix = None
                    for k, v in _ENGINE_SEM_PREFIX.items():
                        if k in str(eng).upper():
                            prefix = v
                            break
                    if prefix is None:
                        continue
                    keep = [w for w in waits if not w.ant_name.startswith(prefix)]
                else:
                    continue
                if keep and len(keep) < len(waits):
                    inst.sync_info = mybir.SyncInfo(
                        on_wait=keep, on_update=si.on_update
                    )

    split_id = 0
    for fn in nc.m.functions:
        for bb in fn.blocks:
            idx = 0
            insts = bb.instructions
            while idx < len(insts):
                inst = insts[idx]
                si = inst.sync_info
                if si is not None and si.on_wait and len(si.on_wait) > 1:
                    waits = list(si.on_wait)
                    inst.sync_info = mybir.SyncInfo(
                        on_wait=[waits[-1]], on_update=si.on_update
                    )
                    for w in waits[:-1]:
                        nd = mybir.InstDrain(
                            name=f"I-waitsplit-{split_id}",
                            ins=[],
                            outs=[],
                            bass_is_fusable=False,
                        )
                        split_id += 1
                        nd.engine = inst.engine
                        nd.sync_info = mybir.SyncInfo(on_wait=[w], on_update=[])
                        insts.insert(idx, nd)
                        idx += 1
                idx += 1


def _finalize(results, pred_pos, edges) -> np.float32:
    # every unordered off-diagonal pair inside the threshold appears exactly
    # once in the device sum -> double it; edge pairs likewise.
    s_all = 0.0
    for r in results:
        s_all += r["acc"].astype(np.float64).sum()
    corr = _edge_correction(pred_pos, edges)
    return np.float32(-2.0 * s_all + 2.0 * corr)


def kernel(pred_pos: np.ndarray, edges: np.ndarray) -> np.ndarray:
    from concourse.bass_utils import run_bass_kernel_spmd

    in_maps, meta = _host_prep(pred_pos)
    nc = _build_program(meta)
    trace = os.environ.get("KERNEL_TRACE", "0") == "1"
    res = run_bass_kernel_spmd(
        nc,
        in_maps,
        core_ids=list(range(NCORES)),
        trace=trace,
    )
    LAST_RESULT["exec_time_ns"] = res.exec_time_ns
    LAST_RESULT["trace"] = res.instructions_and_trace
    LAST_RESULT["meta"] = meta

    return _finalize(res.results, pred_pos, edges)


# revision 27
# speedup vs baseline: 8.2046x; 1.0122x over previous
"""Trainium2 Bass kernel for the pairwise-distance masked log-sum loss.

Reference math (N=8192 points, E=49152 edges):
    dist[i,j] = |p_i - p_j|^2 + 1e-8
    mask      = (dist <= 0.25), edges (both directions) and diagonal zeroed
    loss      = sum(-log(dist) * mask)

Device strategy (8 NeuronCores, SPMD):
  * Points are sorted by x on the host.  Only pairs with |dx| < 0.5 can be
    inside the threshold, so each 128-row tile only needs its own blocks
    (intra-tile pairs) plus a forward window of sorted columns reaching
    x_max(tile) + 0.5.  Pairs outside the window contribute exactly 0.
    Forward windows count each inter-tile pair once; the host doubles that
    partial sum (dist is symmetric).  The 64 row tiles are dealt to the 8
    cores by snake order of window size, so every core runs an identical
    program on identically-shaped inputs (windows padded with far-away
    dummy points, which fall outside the threshold and contribute 0).
  * dist[i,j] = w_i . u_j with K=16 split-precision channels evaluated on
    the TensorE in float32r.  f32r is fp32 rounded to 11 explicit mantissa
    bits (measured round-to-nearest on hardware) at full bf16 PE rate; the
    host splits each coordinate c = ch + cl and |p|^2 = sqh + sql with
    rn11 so every channel is exactly representable and the product sum
    reconstructs |p_i - p_j|^2 + |p|^2-rounding (~1e-6) -- fp32-grade.
  * The diagonal 128x128 block has +10*I accumulated onto it by a second
    matmul (identity lhsT) so ln never sees the dist ~ 1e-8 diagonal.
  * ScalarE computes y = ln(dist) PSUM->SBUF (bf16), VectorE computes
    sum(y * (y <= ln .25)) per partition with the fused
    scalar_tensor_tensor accumulate.
  * Host: loss = -(S_diag + 2*S_windows) + 2*sum(ln dist) over the unique
    non-self edge pairs inside the threshold (the reference masks those
    out, the device sum includes them).
"""

import os

import numpy as np

N = 8192
NCORES = 8
ROW_TILE = 128
TILES = N // ROW_TILE  # 64
SLOTS = TILES // NCORES  # 8 row-tiles per core
KCH = 16  # split-precision channels
COL_CHUNK = 512  # one PSUM bank per matmul
GROUP_COLS = 2048  # ACT/DVE group = 4 PSUM banks
EPS = 1e-8
THR2 = 0.25
XWIN = 0.5
LN_THR = float(np.log(0.25))
DELTA = 6e-6  # positivity cushion folded into the u-side |p|^2 split

USE_FP32R = os.environ.get("KERNEL_FP32R", "1") == "1"
ACC_SLOTS = 64

LAST_RESULT = {}


def _rn11(v: np.ndarray) -> np.ndarray:
    """Round f32/f64 values to 11 explicit mantissa bits (round-to-nearest)
    -- the measured float32r grid."""
    v64 = np.asarray(v, dtype=np.float64)
    m, e = np.frexp(v64)
    q = np.ldexp(np.round(np.ldexp(m, 12)) / (1 << 12), e)
    return q.astype(np.float32)


def _build_channels(pts: np.ndarray):
    """w [16, n] and u [16, n] channel vectors, f32r-grid values, such that
    sum_k w[k,i]*u[k,j] ~= |p_i - p_j|^2 (+eps folded into sql)."""
    c = np.asarray(pts, dtype=np.float32)
    ch = _rn11(c)
    cl = _rn11(c.astype(np.float64) - ch)
    rep = ch.astype(np.float64) + cl  # represented points
    sq = (rep * rep).sum(axis=1)  # f64, exact-ish
    sqh = _rn11(sq)
    sql = _rn11(sq - sqh)
    # u-side |p|^2 carries +DELTA so every pair distance (in particular the
    # diagonal, which is pure split-residual ~ +-2e-6) stays positive for
    # Ln; the diagonal is pushed out of the mask afterwards (ymask).
    squ = sq + DELTA
    squh = _rn11(squ)
    squl = _rn11(squ - squh)

    n = c.shape[0]
    w = np.empty((KCH, n), np.float32)
    u = np.empty((KCH, n), np.float32)
    for a in range(3):
        w[4 * a + 0] = -2.0 * ch[:, a]
        u[4 * a + 0] = ch[:, a]
        w[4 * a + 1] = -2.0 * ch[:, a]
        u[4 * a + 1] = cl[:, a]
        w[4 * a + 2] = -2.0 * cl[:, a]
        u[4 * a + 2] = ch[:, a]
        w[4 * a + 3] = -2.0 * cl[:, a]
        u[4 * a + 3] = cl[:, a]
    w[12] = sqh
    u[12] = 1.0
    w[13] = sql
    u[13] = 1.0
    w[14] = 1.0
    u[14] = squh
    w[15] = 1.0
    u[15] = squl
    return w, u


BANDS = 8


def _host_prep(pred_pos: np.ndarray):
    """Two-level sort (x-bands, y within band), per-tile geometric windows,
    snake balance; build per-core in_maps and program meta.

    Each row tile's window = [its own 128 columns] + every forward column
    that could be within the 0.5 threshold: same/later bands whose x-range
    is reachable, restricted to the tile's y-range +- 0.5.  Every unordered
    off-diagonal pair inside the threshold appears exactly once (own-tile
    lower triangle and diagonal are pushed out of the mask by the +10
    lower-tri matmul); the host doubles the device sum."""
    p = np.asarray(pred_pos, dtype=np.float32)
    per = N // BANDS
    xi = np.argsort(p[:, 0], kind="stable")
    psx = p[xi]
    order_parts = []
    band_x = []
    for b in range(BANDS):
        seg = np.arange(b * per, (b + 1) * per)
        band_x.append(
            (float(psx[seg, 0].min()), float(psx[seg, 0].max()))
        )
        yi = np.argsort(psx[seg, 1], kind="stable")
        order_parts.append(seg[yi])
    order = np.concatenate(order_parts)
    ps = psx[order]
    ys_band = [ps[b * per : (b + 1) * per, 1].astype(np.float64) for b in range(BANDS)]

    w, u = _build_channels(ps)

    CUSH = 1e-3
    tile_ranges = []  # per tile: list of (lo, hi) global column ranges
    for t in range(TILES):
        t0, t1 = t * ROW_TILE, (t + 1) * ROW_TILE
        b = t0 // per
        ya = float(ps[t0:t1, 1].min())
        yb = float(ps[t0:t1, 1].max())
        ranges = [(t0, t1)]  # own tile first (lower-tri masked on device)
        for b2 in range(b, BANDS):
            if band_x[b2][0] - band_x[b][1] >= XWIN - CUSH:
                break
            lo = int(np.searchsorted(ys_band[b2], ya - XWIN - CUSH))
            hi = int(np.searchsorted(ys_band[b2], yb + XWIN + CUSH, side="right"))
            lo += b2 * per
            hi += b2 * per
            if b2 == b:
                lo = max(lo, t1)
            if hi > lo:
                ranges.append((lo, hi))
        tile_ranges.append(ranges)

    widths = [sum(hi - lo for lo, hi in r) for r in tile_ranges]

    # snake-deal tiles (desc width) to cores; slot s width = max in band
    rank = sorted(range(TILES), key=lambda t: -widths[t])
    assign = [[None] * SLOTS for _ in range(NCORES)]
    for s in range(SLOTS):
        band = rank[s * NCORES : (s + 1) * NCORES]
        cores = range(NCORES) if s % 2 == 0 else range(NCORES - 1, -1, -1)
        for t, c in zip(band, cores):
            assign[c][s] = t
    slot_w = []
    for s in range(SLOTS):
        wmax = max(widths[assign[c][s]] for c in range(NCORES))
        slot_w.append(int(np.ceil(wmax / ROW_TILE)) * ROW_TILE)

    # process the smallest slot first (quick PE start while the rest of the
    # input streams in), then descending so the kernel tail is short
    perm = sorted(range(SLOTS), key=lambda s: slot_w[s])
    perm = [perm[0]] + sorted(perm[1:], key=lambda s: -slot_w[s])
    slot_w = [slot_w[s] for s in perm]
    assign = [[assign[c][perm[s]] for s in range(SLOTS)] for c in range(NCORES)]

    # dummy far-away point channels (outside any threshold window)
    _, ud = _build_channels(np.array([[1000.0, 0.0, 0.0]], np.float32))

    # per-core packed input [KCH, SLOTS*128 (rowsW) | windows]
    width = SLOTS * ROW_TILE + sum(slot_w)
    in_maps = []
    import ml_dtypes

    ymask = np.tril(np.full((128, 128), 50.0, np.float32)).astype(ml_dtypes.bfloat16)
    for c in range(NCORES):
        inp = np.empty((KCH, width), np.float32)
        for s in range(SLOTS):
            t = assign[c][s]
            r0 = t * ROW_TILE
            inp[:, s * ROW_TILE : (s + 1) * ROW_TILE] = w[:, r0 : r0 + ROW_TILE]
        off = SLOTS * ROW_TILE
        for s in range(SLOTS):
            t = assign[c][s]
            o = off
            for lo, hi in tile_ranges[t]:
                inp[:, o : o + hi - lo] = u[:, lo:hi]
                o += hi - lo
            if o < off + slot_w[s]:
                inp[:, o : off + slot_w[s]] = ud
            off += slot_w[s]
        in_maps.append({"inp": np.ascontiguousarray(inp), "ymask": ymask})

    meta = {"slot_w": slot_w, "width": width}
    return in_maps, meta


def _edge_correction(pred_pos: np.ndarray, edges: np.ndarray) -> float:
    """sum of ln(dist) over unique unordered non-self edge pairs inside the
    threshold (each such pair appears exactly twice in the device sum)."""
    p = np.asarray(pred_pos, dtype=np.float32)
    e = np.asarray(edges, dtype=np.int64)
    e = e[e[:, 0] != e[:, 1]]
    e = np.sort(e, axis=1)
    e = np.unique(e, axis=0)
    d = p[e[:, 0]] - p[e[:, 1]]
    dist = (d * d).sum(axis=1, dtype=np.float32) + np.float32(EPS)
    m = dist <= np.float32(THR2)
    return float(np.log(dist[m].astype(np.float64)).sum())


def _build_program(meta):
    import concourse.bass as bass
    import concourse.tile as tile
    from concourse import mybir
    from contextlib import ExitStack

    f32 = mybir.dt.float32
    bf16 = mybir.dt.bfloat16
    in_dt = mybir.dt.float32r if USE_FP32R else f32

    slot_w = meta["slot_w"]
    width = meta["width"]

    nc = bass.Bass(
        "TRN2",
        target_bir_lowering=False,
        debug=False,
        num_devices=NCORES,
        enable_asserts=False,
        detect_race_conditions=False,
    )
    inp_d = nc.dram_tensor("inp", [KCH, width], in_dt, kind="ExternalInput").ap()
    ymask_d = nc.dram_tensor("ymask", [128, 128], bf16, kind="ExternalInput").ap()
    acc_d = nc.dram_tensor("acc", [128, ACC_SLOTS], f32, kind="ExternalOutput").ap()

    # how many ACT/DVE groups in total (for y-buffer count: no slot reuse)
    n_groups = sum((wl + GROUP_COLS - 1) // GROUP_COLS for wl in slot_w)
    assert n_groups <= ACC_SLOTS

    with tile.TileContext(nc) as tc, ExitStack() as ctx:
        singles = ctx.enter_context(tc.tile_pool(name="singles", bufs=1))
        psums = ctx.enter_context(tc.tile_pool(name="psums", bufs=2, space="PSUM"))
        ys = ctx.enter_context(tc.tile_pool(name="ys", bufs=n_groups))
        scraps = ctx.enter_context(tc.tile_pool(name="scraps", bufs=2))

        # split the input DMA per slot so PE starts as soon as slot 0's
        # window has landed and later windows stream in under compute
        inp_s = singles.tile([KCH, width], in_dt)
        ymask_s = singles.tile([128, 128], bf16)
        nc.sync.dma_start(out=ymask_s, in_=ymask_d)
        cut = SLOTS * ROW_TILE
        nc.sync.dma_start(out=inp_s[:, :cut], in_=inp_d[:, :cut])
        for s in range(SLOTS):
            eng = nc.sync if s % 2 == 0 else nc.scalar
            eng.dma_start(
                out=inp_s[:, cut : cut + slot_w[s]],
                in_=inp_d[:, cut : cut + slot_w[s]],
            )
            cut += slot_w[s]
        acc_s = singles.tile([128, ACC_SLOTS], f32)

        def reduce_group(psum_t, cols, acc_idx, mask_own):
            y_t = ys.tile([128, GROUP_COLS], bf16, tag="y")
            nc.scalar.activation(
                out=y_t[:, :cols],
                in_=psum_t[:, :cols],
                func=mybir.ActivationFunctionType.Ln,
            )
            if mask_own:
                # +50 on the own-tile lower triangle and diagonal pushes
                # those y values far above the threshold -> excluded
                nc.vector.tensor_tensor(
                    out=y_t[:, :ROW_TILE],
                    in0=y_t[:, :ROW_TILE],
                    in1=ymask_s,
                    op=mybir.AluOpType.add,
                )
            scrap_t = scraps.tile([128, GROUP_COLS], bf16, tag="scrap")
            nc.vector.scalar_tensor_tensor(
                out=scrap_t[:, :cols],
                in0=y_t[:, :cols],
                scalar=LN_THR,
                in1=y_t[:, :cols],
                op0=mybir.AluOpType.is_le,
                op1=mybir.AluOpType.mult,
                accum_out=acc_s[:, acc_idx : acc_idx + 1],
            )

        # Each slot's window = [own 128 cols | forward cols]; the own-tile
        # lower triangle and diagonal are pushed out of the ln-threshold
        # mask by accumulating +10*tril onto psum cols [0, 128).
        acc_idx = 0
        win_off = SLOTS * ROW_TILE
        for s in range(SLOTS):
            lhsT = inp_s[:, s * ROW_TILE : (s + 1) * ROW_TILE]
            wl = slot_w[s]
            done = 0
            while done < wl:
                cols = min(GROUP_COLS, wl - done)
                psum_t = psums.tile([128, GROUP_COLS], f32, tag="ps")
                for k0 in range(0, cols, COL_CHUNK):
                    kw = min(COL_CHUNK, cols - k0)
                    c0 = win_off + done + k0
                    nc.tensor.matmul(
                        out=psum_t[:, k0 : k0 + kw],
                        lhsT=lhsT,
                        rhs=inp_s[:, c0 : c0 + kw],
                        start=True,
                        stop=True,
                    )
                reduce_group(psum_t, cols, acc_idx, done == 0)
                acc_idx += 1
                done += cols
            win_off += wl
        assert acc_idx <= ACC_SLOTS

        nc.sync.dma_start(out=acc_d, in_=acc_s)

    _strip_self_waits(nc, mybir)
    return nc


_SELF_WAIT_OPCODES = {
    "InstMatmult",
    "InstTensorScalarPtr",
    "InstActivation",
    "InstTensorTensor",
    "InstTensorReduce",
    "InstTensorCopy",
    "InstMemset",
}
_ENGINE_SEM_PREFIX = {
    "PE": "PE_",
    "ACT": "Activation_",
    "DVE": "DVE_",
    "POOL": "Pool_",
    "SP": "SP_",
}


def _strip_self_waits(nc, mybir):
    """Walrus caps sync-wait commands per instruction (1 for PE/DVE compute
    structs).  Two post-passes make every instruction single-wait:
      1. drop same-engine self-waits on compute ops (the engines execute
         in order, so an instruction can never run before its same-engine
         predecessors complete; DVE additionally drains between ops), and
         cross-queue DMA-ordering waits on DMACopy (all SBUF regions
         involved here are disjoint);
      2. hoist any remaining extra waits onto same-engine Drain
         instructions inserted just before (an in-order queue enforces
         them for everything after)."""
    for fn in nc.m.functions:
        for bb in fn.blocks:
            for inst in bb.instructions:
                si = inst.sync_info
                if si is None or not si.on_wait or len(si.on_wait) < 2:
                    continue
                tname = type(inst).__name__
                waits = list(si.on_wait)
                if tname == "InstDMACopy":
                    keep = [
                        w
                        for w in waits
                        if not w.ant_name.startswith(("DMAHW", "DMASW"))
                    ]
                elif tname in _SELF_WAIT_OPCODES:
                    eng = getattr(inst.engine, "name", str(inst.engine))
                    prefix = None
                    for k, v in _ENGINE_SEM_PREFIX.items():
                        if k in str(eng).upper():
                            prefix = v
                            break
                    if prefix is None:
                        continue
                    keep = [w for w in waits if not w.ant_name.startswith(prefix)]
                else:
                    continue
                if keep and len(keep) < len(waits):
                    inst.sync_info = mybir.SyncInfo(
                        on_wait=keep, on_update=si.on_update
                    )

    split_id = 0
    for fn in nc.m.functions:
        for bb in fn.blocks:
            idx = 0
            insts = bb.instructions
            while idx < len(insts):
                inst = insts[idx]
                si = inst.sync_info
                if si is not None and si.on_wait and len(si.on_wait) > 1:
                    waits = list(si.on_wait)
                    inst.sync_info = mybir.SyncInfo(
                        on_wait=[waits[-1]], on_update=si.on_update
                    )
                    for w in waits[:-1]:
                        nd = mybir.InstDrain(
                            name=f"I-waitsplit-{split_id}",
                            ins=[],
                            outs=[],
                            bass_is_fusable=False,
                        )
                        split_id += 1
                        nd.engine = inst.engine
                        nd.sync_info = mybir.SyncInfo(on_wait=[w], on_update=[])
                        insts.insert(idx, nd)
                        idx += 1
                idx += 1


def _finalize(results, pred_pos, edges) -> np.float32:
    # every unordered off-diagonal pair inside the threshold appears exactly
    # once in the device sum -> double it; edge pairs likewise.
    s_all = 0.0
    for r in results:
        s_all += r["acc"].astype(np.float64).sum()
    corr = _edge_correction(pred_pos, edges)
    return np.float32(-2.0 * s_all + 2.0 * corr)


def kernel(pred_pos: np.ndarray, edges: np.ndarray) -> np.ndarray:
    from concourse.bass_utils import run_bass_kernel_spmd

    in_maps, meta = _host_prep(pred_pos)
    nc = _build_program(meta)
    trace = os.environ.get("KERNEL_TRACE", "0") == "1"
    res = run_bass_kernel_spmd(
        nc,
        in_maps,
        core_ids=list(range(NCORES)),
        trace=trace,
    )
    LAST_RESULT["exec_time_ns"] = res.exec_time_ns
    LAST_RESULT["trace"] = res.instructions_and_trace
    LAST_RESULT["meta"] = meta

    return _finalize(res.results, pred_pos, edges)
